# revision 9
# baseline (speedup 1.0000x reference)
"""Causal self-attention (B=1, T=4096, D=1024, H=16) on 8 TRN2 NeuronCores.

Sharding: tensor-parallel over heads — 2 heads per core. Each core computes
Q^T/K^T/V for its 2 heads from the full x, runs causal flash-style attention
fully on-chip, applies its slice of the output projection, and writes a
partial [D, T] (transposed) output. The host sums the 8 partials (the
all-reduce of the out projection) and transposes back.

Layout trick: everything is computed in "transposed" space so no on-device
transposes of activations are needed:
  qT/kT: [128, T] with partitions = (head, head_dim)  (via lhsT = w chunk)
  S^T tile: [128 kv, 512 q] = kT_slice.T-matmul  (kv on partitions)
  p^T = exp(S^T/8) (ACT), causal mask by elementwise multiply
  y^T: [65, 512] accumulated in PSUM via lhsT = [v | 1] (ones col => rowsum)
  outT: [128 d, 512 t] via lhsT = w_out chunk, rhs = y^T stacked
V is needed in natural [t, dh] layout (contraction over kv partitions), so it
is computed as v^T like q/k and transposed on the PE (only 32 transposes).

The emission interleaves projection t-slice ts with attention q-tile i=ts
(whose kv window only needs slices <= ts) so all engines ramp together.

Matmuls use float32r (TF32-like, full-rate for moving dim >= 256).
"""

import numpy as np

T = 4096
D = 1024
H = 16
DH = 64
NCORES = 8
HPC = H // NCORES          # heads per core = 2
CD = HPC * DH              # per-core hidden slice = 128
QT = 512                   # query tile (free dim of S^T matmuls)
KT = 128                   # kv tile (partition dim of S^T)
NQ = T // QT               # 8 big q tiles
TS = 512                   # phase-1 t-slice
NTS = T // TS              # 8 slices
NKC = D // 128             # 8 contraction chunks of d_model

_CACHE = {}


def _build():
    import concourse.bass as bass
    import concourse.tile as tile
    from concourse import bacc, mybir
    from concourse.masks import make_identity

    F32 = mybir.dt.float32
    F32R = mybir.dt.float32r
    AF = mybir.ActivationFunctionType

    nc = bacc.Bacc("TRN2", target_bir_lowering=False, debug=False,
                   num_devices=NCORES)

    xT_d = nc.dram_tensor("xt", [D + 1, T], F32R, kind="ExternalInput").ap()
    wq_d = nc.dram_tensor("wq", [D + 1, CD], F32R, kind="ExternalInput").ap()
    wk_d = nc.dram_tensor("wk", [D + 1, CD], F32R, kind="ExternalInput").ap()
    wv_d = nc.dram_tensor("wv", [D + 1, CD], F32R, kind="ExternalInput").ap()
    wo_d = nc.dram_tensor("wo", [CD, D], F32R, kind="ExternalInput").ap()
    bo_d = nc.dram_tensor("bo", [NKC, 128], F32, kind="ExternalInput").ap()
    outT_d = nc.dram_tensor("outt", [D, T], F32R, kind="ExternalOutput").ap()

    with (
        tile.TileContext(nc) as tc,
        tc.tile_pool(name="persist", bufs=1) as persist,
        tc.tile_pool(name="xt", bufs=2) as xtp,
        tc.tile_pool(name="vtq", bufs=2) as vtqp,
        tc.tile_pool(name="pt", bufs=4) as ptp,
        tc.tile_pool(name="rs", bufs=2) as rsp,
        tc.tile_pool(name="rsb", bufs=2) as rsbp,
        tc.tile_pool(name="ot", bufs=3) as otp,
        tc.tile_pool(name="ps_sg", bufs=2, space="PSUM") as psg,
        tc.tile_pool(name="ps_y", bufs=2, space="PSUM") as psy,
        tc.tile_pool(name="ps_m", bufs=2, space="PSUM") as psm,
    ):
        # ---------------- constants & persistent tiles ----------------
        ones64 = persist.tile([1, 64], F32R)
        ones64_f = persist.tile([1, 64], F32)
        nc.vector.memset(ones64_f, 1.0)
        nc.vector.tensor_copy(out=ones64, in_=ones64_f)
        # shifted causal masks: keep (kv_p <= q_f - 128*jr)
        masks = persist.tile([128, 4, QT], F32)
        nc.vector.memset(masks, 1.0)
        for jr in range(4):
            nc.gpsimd.affine_select(
                out=masks[:, jr, :], in_=masks[:, jr, :],
                compare_op=mybir.AluOpType.is_ge, fill=0.0,
                base=-128 * jr, pattern=[[1, QT]], channel_multiplier=-1,
            )
        bo_sb = persist.tile([128, NKC], F32)
        nc.sync.dma_start(out=bo_sb, in_=bo_d.rearrange("a p -> p a"))
        wo_sb = persist.tile([128, NKC, 128], F32R)
        nc.sync.dma_start(out=wo_sb, in_=wo_d.rearrange("p (a m) -> p a m", a=NKC))
        ident = persist.tile([128, 128], F32)
        make_identity(nc, ident)

        # projection weights: [128, chunk, CD] (+ bias row separately)
        wq_sb = persist.tile([128, NKC, CD], F32R)
        wk_sb = persist.tile([128, NKC, CD], F32R)
        wv_sb = persist.tile([128, NKC, CD], F32R)
        nc.sync.dma_start(out=wq_sb, in_=wq_d[0:D, :].rearrange("(a p) m -> p a m", p=128))
        nc.sync.dma_start(out=wk_sb, in_=wk_d[0:D, :].rearrange("(a p) m -> p a m", p=128))
        nc.sync.dma_start(out=wv_sb, in_=wv_d[0:D, :].rearrange("(a p) m -> p a m", p=128))
        wq1 = persist.tile([1, CD], F32R)
        wk1 = persist.tile([1, CD], F32R)
        wv1 = persist.tile([1, CD], F32R)
        nc.sync.dma_start(out=wq1, in_=wq_d[D:D + 1, :])
        nc.sync.dma_start(out=wk1, in_=wk_d[D:D + 1, :])
        nc.sync.dma_start(out=wv1, in_=wv_d[D:D + 1, :])

        # persistent activations
        qTs = persist.tile([128, T], F32R)   # rows h*64+dh
        kTs = persist.tile([128, T], F32R)
        yTs = persist.tile([128, T], F32R)
        # v natural, both heads, with a ones column per head:
        # free layout [32 kv-tiles, 130]: cols 0:64 v_h0, 64 ones, 65:129 v_h1, 129 ones
        v_sb = persist.tile([128, T // KT, 130], F32R)
        vones_f = persist.tile([128, T // KT, 1], F32)
        nc.vector.memset(vones_f, 1.0)
        nc.vector.tensor_copy(out=v_sb[:, :, 64:65], in_=vones_f)
        nc.vector.tensor_copy(out=v_sb[:, :, 129:130], in_=vones_f)

        def proj_slice(ts):
            sl = slice(ts * TS, (ts + 1) * TS)
            xt_e = xtp.tile([128, NKC, TS], F32R, tag="xt", name=f"xt{ts}")
            for k in range(NKC):
                nc.sync.dma_start(
                    out=xt_e[:, k, :],
                    in_=xT_d[k * 128:(k + 1) * 128, sl])
            xt1 = xtp.tile([1, TS], F32R, tag="xt1", name=f"xt1_{ts}")
            nc.sync.dma_start(out=xt1, in_=xT_d[D:D + 1, sl])

            for which, w_sb, w1, dest in (
                    ("q", wq_sb, wq1, qTs), ("k", wk_sb, wk1, kTs),
                    ("v", wv_sb, wv1, None)):
                ps = psm.tile([128, TS], F32, tag="m", name=f"p{which}{ts}")
                for k in range(NKC):
                    nc.tensor.matmul(ps, w_sb[:, k, :], xt_e[:, k, :],
                                     start=(k == 0), stop=False)
                nc.tensor.matmul(ps, w1, xt1, start=False, stop=True)
                if dest is not None:
                    nc.vector.tensor_copy(out=dest[:, sl], in_=ps)
                else:
                    vt_q = vtqp.tile([128, TS], F32, tag="vtq", name=f"vtq{ts}")
                    nc.vector.tensor_copy(out=vt_q, in_=ps)
                    for jj in range(TS // KT):
                        j = ts * (TS // KT) + jj
                        pt_ps = psm.tile([128, 128], F32, tag="m",
                                         name=f"tr{j}")
                        nc.tensor.transpose(
                            pt_ps, vt_q[:, jj * 128:(jj + 1) * 128], ident)
                        nc.vector.tensor_copy(out=v_sb[:, j, 0:64],
                                              in_=pt_ps[:, 0:64])
                        nc.vector.tensor_copy(out=v_sb[:, j, 65:129],
                                              in_=pt_ps[:, 64:128])

        def attention(i):
            qsl = slice(i * QT, (i + 1) * QT)
            nj = (i + 1) * (QT // KT)           # kv tiles for this q tile
            ys = [psy.tile([65, QT], F32, tag="y", name=f"y{i}h{h}")
                  for h in range(HPC)]
            for g in range(nj // 2):
                pts = []
                for h in range(HPC):
                    hs = slice(h * DH, (h + 1) * DH)
                    sg = psg.tile([128, 2, QT], F32, tag="sg",
                                  name=f"sg{i}g{g}h{h}")
                    for jj in range(2):
                        j = 2 * g + jj
                        nc.tensor.matmul(
                            sg[:, jj, :],
                            kTs[hs, j * KT:(j + 1) * KT],
                            qTs[hs, qsl], start=True, stop=True)
                    pt = ptp.tile([128, 2, QT], F32R, tag="pt",
                                  name=f"pt{i}g{g}h{h}")
                    nc.scalar.activation(out=pt, in_=sg, func=AF.Exp,
                                         scale=0.125)
                    pts.append(pt)
                for h in range(HPC):
                    pt = pts[h]
                    for jj in range(2):
                        j = 2 * g + jj
                        jr = j - (QT // KT) * i
                        if jr >= 0:   # diagonal tile: apply causal mask
                            nc.vector.tensor_mul(
                                out=pt[:, jj, :], in0=pt[:, jj, :],
                                in1=masks[:, jr, :])
                        f0 = max(0, 128 * jr)
                        nc.tensor.matmul(
                            ys[h][:, f0:], v_sb[:, j, 65 * h:65 * h + 65],
                            pt[:, jj, f0:], start=(j == 0),
                            stop=(j == nj - 1))
            # normalize: y[0:64] / y[64] -> yTs
            for h in range(HPC):
                rs = rsp.tile([1, QT], F32R, tag="rs", name=f"rs{i}h{h}")
                with nc.allow_low_precision(reason="tf32-grade kernel"):
                    nc.vector.reciprocal(out=rs, in_=ys[h][64:65, :])
                rb_ps = psm.tile([64, QT], F32, tag="m", name=f"rb{i}h{h}")
                nc.tensor.matmul(rb_ps, ones64, rs, start=True, stop=True)
                rsb = rsbp.tile([64, QT], F32R, tag="rsb", name=f"rsb{i}h{h}")
                nc.vector.tensor_copy(out=rsb, in_=rb_ps)
                nc.vector.tensor_mul(
                    out=yTs[h * DH:(h + 1) * DH, qsl],
                    in0=ys[h][0:64, :], in1=rsb)
            # out projection for this t slice
            for d in range(NKC):
                po = psm.tile([128, QT], F32, tag="m", name=f"po{i}d{d}")
                nc.tensor.matmul(po, wo_sb[:, d, :], yTs[:, qsl],
                                 start=True, stop=True)
                ot = otp.tile([128, QT], F32R, tag="ot", name=f"ot{i}d{d}")
                nc.vector.tensor_scalar_add(out=ot, in0=po,
                                            scalar1=bo_sb[:, d:d + 1])
                nc.sync.dma_start(
                    out=outT_d[d * 128:(d + 1) * 128, qsl], in_=ot)

        for ts in range(NTS):
            proj_slice(ts)
            attention(ts)

    nc.compile()
    return nc


def _prep_inputs(x, w_qkv, b_qkv, w_out, b_out):
    x = np.ascontiguousarray(np.asarray(x, dtype=np.float32).reshape(T, D))
    w_qkv = np.asarray(w_qkv, dtype=np.float32)
    b_qkv = np.asarray(b_qkv, dtype=np.float32)
    w_out = np.asarray(w_out, dtype=np.float32)
    b_out = np.asarray(b_out, dtype=np.float32)

    xT = np.empty((D + 1, T), np.float32)
    xT[:D] = x.T
    xT[D] = 1.0
    xT = np.ascontiguousarray(xT)

    in_maps = []
    for c in range(NCORES):
        h0 = HPC * c
        cols = np.arange(h0 * DH, (h0 + HPC) * DH)
        m = {"xt": xT}
        for name, off in (("wq", 0), ("wk", D), ("wv", 2 * D)):
            w = np.empty((D + 1, CD), np.float32)
            w[:D] = w_qkv[:, off + cols]
            w[D] = b_qkv[off + cols]
            m[name] = w
        m["wo"] = np.ascontiguousarray(w_out[cols, :])
        bo = b_out if c == 0 else np.zeros_like(b_out)
        m["bo"] = np.ascontiguousarray(bo.reshape(NKC, 128))
        in_maps.append(m)
    return in_maps


def kernel(x, w_qkv, b_qkv, w_out, b_out, _trace=False):
    from concourse.bass_utils import run_bass_kernel_spmd

    if "nc" not in _CACHE:
        _CACHE["nc"] = _build()
    nc = _CACHE["nc"]

    in_maps = _prep_inputs(x, w_qkv, b_qkv, w_out, b_out)
    res = run_bass_kernel_spmd(nc, in_maps, core_ids=list(range(NCORES)),
                               trace=_trace)
    _CACHE["last_result"] = res
    acc = res.results[0]["outt"].astype(np.float32)
    for c in range(1, NCORES):
        acc = acc + res.results[c]["outt"]
    return np.ascontiguousarray(acc.T).reshape(1, T, D)
